# revision 11
# baseline (speedup 1.0000x reference)
"""Trainium2 Bass kernel for nn_Mlp_moe: dense patch-token MLP + top-1 gated
atom (expert) routing for 6 CLS task tokens.

Sharding over 8 NeuronCores:
  - Patch MLP: data-parallel over batch B=64 -> 8 batches (1568 patch tokens)
    per core. MLP weights replicated (SBUF-resident, bf16).
  - Atom/CLS part: hidden dim H=3072 sharded 8-way (384 per core); every core
    processes all 384 CLS tokens on its H-shard and emits a partial output
    summed on the host. Routing (gate logits/sigmoid/top-1 masks) is computed
    on the host (O(B*6*D), negligible) and shipped as {0,1}/weight masks
    folded into the device compute.

Key structure (v2):
  - CLS tokens are ordered slot-major (t = slot*64 + batch). Each slot has
    exactly TWO candidate atoms (left LEFT_KEYS//5, right RIGHT_KEYS//5), so
    the atom in/out GEMMs only compute 2 candidates per token (768 moving
    cols per d-tile instead of 1920): atoms 0-2 cover contiguous 128-col
    slot pairs, atoms 3/4 cover strided 3x64 slot sets.
  - PE order: patch chunk0 (piecewise w1 arrival) -> atom in-GEMM+select ->
    chunk1 -> chunk2 -> atom out-GEMM -> chunk3. Atom phases sit in the DMA
    shadow. Warm-up matmuls run during the DMA boot window so the PE p-state
    is ramped when real work arrives.
  - DMA: sync queue carries only the patch critical path (x0, w1 pieces, b1,
    x1); the gpsimd queue (cls/atom tensors, w2, x2/3) is held back behind a
    dummy Pool-engine dependency so it does not steal bandwidth from w1;
    vector queue carries all output stores.
"""

import numpy as np
import ml_dtypes

import concourse.bass as bass
import concourse.bacc as bacc
import concourse.mybir as mybir
from concourse import tile
from concourse.bass_utils import run_bass_kernel_spmd

NCORES = 8
B, NCLS, P, D, H = 64, 6, 196, 768, 3072
NA = 5
HSH = H // NCORES            # 384: per-core atom hidden shard
BPC = B // NCORES            # 8 batches per core
TPC = BPC * P                # 1568 patch tokens per core
NT = B * NCLS                # 384 cls tokens
DT = D // 128                # 6 d-tiles
HT = H // 128                # 24 h-tiles
HLT = NA * HSH // 128        # 15 atom h-shard tiles (a-major, 3 per atom)
KPA = HSH // 128             # 3 h-shard tiles per atom
CW = 392
NCH = 4

W1PIECES = [2, 2, 3, 3, 4, 4, 6]       # h-tiles per w1 load piece
W1QUEUE = [0, 1, 0, 1, 0, 1, 0]        # 0=sync, 1=scalar (alternating)
W1OFF = np.cumsum([0] + W1PIECES)      # [0,2,4,7,10,14,18,24]
PIECE_OF_H = sum(([i] * n for i, n in enumerate(W1PIECES)), [])

LEFT_KEYS = np.array([3, 4, 8, 9, 13, 14], dtype=np.int64)
RIGHT_KEYS = np.array([15, 20, 16, 21, 17, 22], dtype=np.int64)

BF16 = mybir.dt.bfloat16
F32 = mybir.dt.float32
AF = mybir.ActivationFunctionType

_CACHE = {}
LAST_RESULTS = None  # BassKernelResults of the most recent run (for profiling)


def _build_program():
    nc = bacc.Bacc(None, target_bir_lowering=False, debug=False,
                   num_devices=NCORES)

    # partition-major packed inputs (see host layouts in kernel())
    xT_d = nc.dram_tensor("xT", [128, NCH * DT * CW], BF16,
                          kind="ExternalInput")
    w1T_d = nc.dram_tensor("w1T", [128, HT * DT * 128], BF16,
                           kind="ExternalInput")
    b1T_d = nc.dram_tensor("b1T", [128, HT], F32, kind="ExternalInput")
    w2T_d = nc.dram_tensor("w2T", [128, HT * D], BF16, kind="ExternalInput")
    clsT_d = nc.dram_tensor("clsT", [128, DT * NT], BF16,
                            kind="ExternalInput")
    ainT_d = nc.dram_tensor("ainT", [DT, 128, NA * HSH], BF16,
                            kind="ExternalInput")
    ainbT_d = nc.dram_tensor("ainbT", [128, HLT], F32, kind="ExternalInput")
    aoutT_d = nc.dram_tensor("aoutT", [NA, 128, KPA * D], BF16,
                             kind="ExternalInput")
    masks_d = nc.dram_tensor("masks", [128, 4 * NT], BF16,
                             kind="ExternalInput")
    poutT_d = nc.dram_tensor("poutT", [DT, 128, TPC], F32,
                             kind="ExternalOutput")
    cpartT_d = nc.dram_tensor("cpartT", [DT, 128, NT], F32,
                              kind="ExternalOutput")

    with tile.TileContext(nc) as tc:
        with (
            tc.tile_pool(name="w", bufs=1) as wp,
            tc.tile_pool(name="gat", bufs=1) as gp,
            tc.tile_pool(name="sel", bufs=1) as sp,
            tc.tile_pool(name="xin", bufs=4) as xp,
            tc.tile_pool(name="g1", bufs=24) as g1p,
            tc.tile_pool(name="ostg", bufs=3) as op,
            tc.tile_pool(name="ps", bufs=8, space="PSUM") as pp,
        ):
            # ---- patch critical path split over sync+scalar queues (each
            # queue sustains only ~220 B/ns), in consumption order ----
            xs = [xp.tile([128, DT * CW], BF16, tag="x", name="x")]
            nc.sync.dma_start(xs[0][:, :3 * CW], xT_d[:, :3 * CW])
            nc.scalar.dma_start(xs[0][:, 3 * CW:], xT_d[:, 3 * CW:DT * CW])
            b1T = wp.tile([128, HT], F32, tag="b1", name="b1")
            nc.scalar.dma_start(b1T[:], b1T_d[:])
            w1p = []
            for i, n in enumerate(W1PIECES):
                t = wp.tile([128, n * DT * 128], BF16, tag=f"w1p{i}",
                            name=f"w1p{i}")
                eng = nc.sync if W1QUEUE[i] == 0 else nc.scalar
                eng.dma_start(
                    t[:], w1T_d[:, W1OFF[i] * DT * 128:W1OFF[i + 1] * DT * 128])
                w1p.append(t)
            xs.append(xp.tile([128, DT * CW], BF16, tag="x", name="x"))
            nc.sync.dma_start(xs[1][:], xT_d[:, DT * CW:2 * DT * CW])

            # ---- warm-up: ramp the PE p-state during the DMA boot window.
            # Independent psum tiles so the matmuls stream back-to-back. ----
            wtile = wp.tile([128, 512], BF16, tag="warm", name="warm")
            nc.gpsimd.memset(wtile[:], 0)
            for i in range(10):
                wps = pp.tile([128, 512], F32, tag="ps", name=f"wps{i}")
                nc.tensor.matmul(wps[:, :512], wtile[:, :128], wtile[:, :512],
                                 start=True, stop=True)

            # ---- gpsimd queue: atom + later tensors, held behind chunk0's
            # first gelu so it does not compete with the w1 stream ----
            junk = wp.tile([128, CW], BF16, tag="junk", name="junk")
            gate_src = {}  # filled by patch_chunk(0): first g1 tile

            def gated_dma(t, src):
                # Pre-write the destination so the DMA carries a real data
                # dependency on the gate (the scheduler reorders engine
                # instructions by data deps, not program order).
                nc.gpsimd.tensor_copy(t[:, :64], junk[:, :64])
                nc.gpsimd.dma_start(t[:], src)

            def gpsimd_loads():
                nc.gpsimd.tensor_copy(junk[:], gate_src[0][:, :CW])
                w2T = [wp.tile([128, 12 * D], BF16, tag=f"w2{q}",
                               name=f"w2{q}") for q in range(2)]
                gated_dma(w2T[0], w2T_d[:, :12 * D])
                gated_dma(w2T[1], w2T_d[:, 12 * D:])
                clsT = wp.tile([128, DT * NT], BF16, tag="cls", name="cls")
                gated_dma(clsT, clsT_d[:])
                ainT = [wp.tile([128, NA * HSH], BF16, tag=f"ain{d}",
                                name=f"ain{d}") for d in range(DT)]
                for d in range(DT):
                    gated_dma(ainT[d], ainT_d[d])
                ainbT = wp.tile([128, HLT], F32, tag="ainb", name="ainb")
                nc.gpsimd.dma_start(ainbT[:], ainbT_d[:])
                masks = wp.tile([128, 4 * NT], BF16, tag="mask", name="mask")
                gated_dma(masks, masks_d[:])
                xs.append(load_x2(2))
                xs.append(load_x2(3))
                aoutT = [wp.tile([128, KPA * D], BF16, tag=f"ao{a}",
                                 name=f"ao{a}") for a in range(NA)]
                for a in range(NA):
                    gated_dma(aoutT[a], aoutT_d[a])
                return clsT, ainT, w2T, ainbT, masks, aoutT

            def load_x2(ci):
                xa = xp.tile([128, DT * CW], BF16, tag="x", name="x")
                nc.gpsimd.tensor_copy(xa[:, :64], junk[:, :64])
                nc.gpsimd.dma_start(
                    xa[:], xT_d[:, ci * DT * CW:(ci + 1) * DT * CW])
                return xa

            # ---- patch chunk: in-GEMM -> gelu -> out-GEMM -> store ----
            def patch_chunk(ci, xa, w2T, split_last=False):
                g1s = []
                for h in range(HT):
                    pi = PIECE_OF_H[h]
                    hh = h - W1OFF[pi]
                    ps = pp.tile([128, 512], F32, tag="ps", name="ps")
                    for d in range(DT):
                        c0 = (hh * DT + d) * 128
                        nc.tensor.matmul(ps[:, :CW], w1p[pi][:, c0:c0 + 128],
                                         xa[:, d * CW:(d + 1) * CW],
                                         start=(d == 0), stop=(d == DT - 1))
                    g1 = g1p.tile([128, CW], BF16, tag="g1", name="g1")
                    nc.scalar.activation(g1[:], ps[:, :CW], AF.Gelu,
                                         bias=b1T[:, h:h + 1])
                    g1s.append(g1)
                    if ci == 0 and h == 0:
                        gate_src[0] = g1
                        gate_src["atom"] = gpsimd_loads()
                if w2T is None:
                    w2T = gate_src["atom"][2]
                for dp in range(DT):
                    halves = [(0, CW)]
                    for (o, w) in halves:
                        ps = pp.tile([128, 512], F32, tag="ps", name="ps")
                        for h in range(HT):
                            c0 = (h % 12) * D + dp * 128
                            nc.tensor.matmul(ps[:, :w],
                                             w2T[h // 12][:, c0:c0 + 128],
                                             g1s[h][:, o:o + w],
                                             start=(h == 0), stop=(h == HT - 1))
                        stg = op.tile([128, CW], F32, tag="ostg", name="ostg")
                        nc.vector.tensor_copy(stg[:, :w], ps[:, :w])
                        nc.sync.dma_start(
                            poutT_d[dp][:, ci * CW + o:ci * CW + o + w],
                            stg[:, :w])

            patch_chunk(0, xs[0], None)
            clsT, ainT, w2T, ainbT, masks, aoutT = gate_src["atom"]

            # ---- atom in-GEMM + gelu (2 candidates per slot) ----
            # atoms 0-2: slot pair (2a, 2a+1) -> cols [a*128, (a+1)*128)
            # atoms 3/4: slots (0,2,4)/(1,3,5) -> strided 3x64 col blocks
            cls_r = clsT[:].rearrange("p (d s b) -> p d s b", d=DT, s=NCLS)
            gL = [gp.tile([128, NT], BF16, tag=f"gL{k}", name=f"gL{k}")
                  for k in range(KPA)]
            gR = [gp.tile([128, NT], BF16, tag=f"gR{k}", name=f"gR{k}")
                  for k in range(KPA)]
            for a in range(NA):
                for k in range(KPA):
                    ps = pp.tile([128, 512], F32, tag="ps", name="ps")
                    wcol = a * HSH + k * 128
                    for d in range(DT):
                        if a < 3:
                            mov = clsT[:, d * NT + a * 128:
                                       d * NT + (a + 1) * 128]
                            out = ps[:, :128]
                        else:
                            mov = cls_r[:, d, (a - 3):NCLS:2, :]
                            out = ps[:, :192].rearrange("p (c b) -> p c b",
                                                        c=3)
                        nc.tensor.matmul(out, ainT[d][:, wcol:wcol + 128],
                                         mov, start=(d == 0),
                                         stop=(d == DT - 1))
                    hl = a * KPA + k
                    if a < 3:
                        oap = gL[k][:, a * 128:(a + 1) * 128]
                        iap = ps[:, :128]
                    else:
                        oap = gR[k][:].rearrange(
                            "p (s b) -> p s b", s=NCLS)[:, (a - 3):NCLS:2, :]
                        iap = ps[:, :192].rearrange("p (c b) -> p c b", c=3)
                    nc.scalar.activation(oap, iap, AF.Gelu,
                                         bias=ainbT[:, hl:hl + 1])

            # ---- select + dst-weight masks (DVE, overlaps patch) ----
            mL, mR = masks[:, :NT], masks[:, NT:2 * NT]
            mwL, mwR = masks[:, 2 * NT:3 * NT], masks[:, 3 * NT:]
            hLs, hRs = [], []
            for k in range(KPA):
                t1 = sp.tile([128, NT], BF16, tag="t1", name="t1")
                t2 = sp.tile([128, NT], BF16, tag="t2", name="t2")
                sel = sp.tile([128, NT], BF16, tag=f"sel{k}", name=f"sel{k}")
                nc.vector.tensor_mul(t1[:], gL[k][:], mL)
                nc.vector.tensor_mul(t2[:], gR[k][:], mR)
                nc.vector.tensor_add(sel[:], t1[:], t2[:])
                hL = sp.tile([128, NT], BF16, tag=f"hL{k}", name=f"hL{k}")
                hR = sp.tile([128, NT], BF16, tag=f"hR{k}", name=f"hR{k}")
                nc.vector.tensor_mul(hL[:], sel[:], mwL)
                nc.vector.tensor_mul(hR[:], sel[:], mwR)
                hLs.append(hL)
                hRs.append(hR)

            patch_chunk(1, xs[1], w2T)
            patch_chunk(2, xs[2], w2T)

            # ---- atom out-GEMM: psL (atoms 0-2) + psR (atoms 3/4) ----
            for dp in range(DT):
                psL = pp.tile([128, 512], F32, tag="ps", name="ps")
                psR = pp.tile([128, 512], F32, tag="ps", name="ps")
                for a in range(3):
                    for k in range(KPA):
                        c0 = k * D + dp * 128
                        nc.tensor.matmul(psL[:, a * 128:(a + 1) * 128],
                                         aoutT[a][:, c0:c0 + 128],
                                         hLs[k][:, a * 128:(a + 1) * 128],
                                         start=(k == 0), stop=(k == KPA - 1))
                for a in (3, 4):
                    for k in range(KPA):
                        c0 = k * D + dp * 128
                        oap = psR[:, :NT].rearrange(
                            "p (s b) -> p s b", s=NCLS)[:, (a - 3):NCLS:2, :]
                        mov = hRs[k][:].rearrange(
                            "p (s b) -> p s b", s=NCLS)[:, (a - 3):NCLS:2, :]
                        nc.tensor.matmul(oap, aoutT[a][:, c0:c0 + 128], mov,
                                         start=(k == 0), stop=(k == KPA - 1))
                stg = op.tile([128, CW], F32, tag="ostg", name="ostg")
                nc.vector.tensor_copy(stg[:, :NT], psL[:, :NT])
                nc.vector.tensor_add(stg[:, :NT], stg[:, :NT], psR[:, :NT])
                nc.sync.dma_start(cpartT_d[dp], stg[:, :NT])

            patch_chunk(3, xs[3], w2T, split_last=True)

    nc.compile()
    return nc


def _sigmoid(x):
    out = np.empty_like(x)
    pos = x >= 0
    out[pos] = 1.0 / (1.0 + np.exp(-x[pos]))
    ex = np.exp(x[~pos])
    out[~pos] = ex / (1.0 + ex)
    return out


def kernel(x, patch_w1, patch_b1, patch_w2, patch_b2, gate_delta,
           atom_in_w, atom_in_b, atom_out_w, atom_out_b):
    x = np.asarray(x, dtype=np.float32)
    patch_w1 = np.asarray(patch_w1, dtype=np.float32)
    patch_b1 = np.asarray(patch_b1, dtype=np.float32)
    patch_w2 = np.asarray(patch_w2, dtype=np.float32)
    patch_b2 = np.asarray(patch_b2, dtype=np.float32)
    gate_delta = np.asarray(gate_delta, dtype=np.float32)
    atom_in_w = np.asarray(atom_in_w, dtype=np.float32)
    atom_in_b = np.asarray(atom_in_b, dtype=np.float32)
    atom_out_w = np.asarray(atom_out_w, dtype=np.float32)
    atom_out_b = np.asarray(atom_out_b, dtype=np.float32)

    bf = ml_dtypes.bfloat16

    # ---- host routing (tiny); slot-major token order t = n*64 + b ----
    cls3 = x[:, :NCLS, :]                                   # [B, 6, D]
    logits = np.einsum("bnd,nd->bn", cls3, gate_delta)      # [B, 6] f32
    choose_left = logits >= 0
    p_left = _sigmoid(logits)
    wgt = np.where(choose_left, p_left, 1.0 - p_left).astype(np.float32)
    keys = np.where(choose_left, LEFT_KEYS[None, :], RIGHT_KEYS[None, :])
    # slot-major flattening
    left_sm = choose_left.T.reshape(-1)                     # [384]
    w_sm = wgt.T.reshape(-1).astype(np.float32)             # [384]
    dst_sm = (keys % NA).T.reshape(-1)                      # [384]

    mL = left_sm.astype(np.float32)
    mR = 1.0 - mL
    mwL = mR * w_sm          # dst in {0,1,2} <=> right-chosen
    mwR = mL * w_sm          # dst in {3,4}   <=> left-chosen
    masks = np.ascontiguousarray(np.broadcast_to(
        np.concatenate([mL, mR, mwL, mwR]).reshape(1, 4 * NT),
        (128, 4 * NT))).astype(bf)

    # ---- replicated tensors (partition-major packed) ----
    # clsT[p, d*NT + n*64 + b] = cls3[b, n, d*128+p]
    clsT = np.ascontiguousarray(
        cls3.reshape(B, NCLS, DT, 128).transpose(3, 2, 1, 0)
    ).reshape(128, DT * NT).astype(bf)
    # w1T[p, (h*DT + d)*128 + m] = patch_w1[h*128+m, d*128+p]
    w1T = np.ascontiguousarray(
        patch_w1.reshape(HT, 128, DT, 128).transpose(3, 0, 2, 1)
    ).reshape(128, HT * DT * 128).astype(bf)
    b1T = np.ascontiguousarray(patch_b1.reshape(HT, 128).T)
    # w2T[p, h*D + dp*128 + m] = patch_w2[dp*128+m, h*128+p]
    w2T = np.ascontiguousarray(
        patch_w2.reshape(DT, 128, HT, 128).transpose(3, 2, 0, 1)
    ).reshape(128, HT * D).astype(bf)

    # ---- per-core tensors ----
    patch = x[:, NCLS:, :].reshape(NCORES, TPC, D)
    # xT[p, ci*DT*CW + d*CW + t] = patch[c][ci*CW+t, d*128+p]
    xT_all = np.ascontiguousarray(
        patch.reshape(NCORES, NCH, CW, DT, 128).transpose(0, 4, 1, 3, 2)
    ).reshape(NCORES, 128, NCH * DT * CW).astype(bf)

    ainT_all, ainbT_all, aoutT_all = [], [], []
    for c in range(NCORES):
        hsl = slice(HSH * c, HSH * (c + 1))
        # ainT[d, p, a*HSH + k*128 + m] = atom_in_w[a, hsl0 + k*128+m, d*128+p]
        ainT = np.ascontiguousarray(
            atom_in_w[:, hsl, :].reshape(NA, KPA, 128, DT, 128)
            .transpose(3, 4, 0, 1, 2)).reshape(DT, 128, NA * HSH).astype(bf)
        ainT_all.append(ainT)
        ainbT_all.append(np.ascontiguousarray(
            atom_in_b[:, hsl].reshape(HLT, 128).T))
        # aoutT[a, p, k*D + dp*128 + m] = atom_out_w[a, dp*128+m, hsl0+k*128+p]
        aoutT = np.ascontiguousarray(
            atom_out_w[:, :, hsl].reshape(NA, DT, 128, KPA, 128)
            .transpose(0, 4, 3, 1, 2)).reshape(NA, 128, KPA * D).astype(bf)
        aoutT_all.append(aoutT)

    in_maps = []
    for c in range(NCORES):
        in_maps.append({
            "xT": xT_all[c], "w1T": w1T, "b1T": b1T, "w2T": w2T,
            "clsT": clsT, "ainT": ainT_all[c], "ainbT": ainbT_all[c],
            "aoutT": aoutT_all[c], "masks": masks,
        })

    nc = _CACHE.get("nc")
    if nc is None:
        nc = _build_program()
        _CACHE["nc"] = nc

    res = run_bass_kernel_spmd(nc, in_maps, core_ids=list(range(NCORES)))
    global LAST_RESULTS
    LAST_RESULTS = res

    # ---- host gather ----
    patch_out = np.empty((B, P, D), dtype=np.float32)
    for c in range(NCORES):
        poutT = res.results[c]["poutT"].reshape(D, TPC)
        patch_out[BPC * c:BPC * (c + 1)] = (
            poutT.T + patch_b2[None, :]).reshape(BPC, P, D)

    cpart = np.zeros((D, NT), dtype=np.float32)
    for c in range(NCORES):
        cpart += res.results[c]["cpartT"].reshape(D, NT)
    cls_sm = cpart.T + w_sm[:, None] * atom_out_b[dst_sm, :]   # [384, D]
    cls_out = cls_sm.reshape(NCLS, B, D).transpose(1, 0, 2)

    return np.concatenate([cls_out, patch_out], axis=1)


# revision 12
# speedup vs baseline: 1.0053x; 1.0053x over previous
"""Trainium2 Bass kernel for nn_Mlp_moe: dense patch-token MLP + top-1 gated
atom (expert) routing for 6 CLS task tokens.

Sharding over 8 NeuronCores:
  - Patch MLP: data-parallel over batch B=64 -> 8 batches (1568 patch tokens)
    per core. MLP weights replicated (SBUF-resident, bf16).
  - Atom/CLS part: hidden dim H=3072 sharded 8-way (384 per core); every core
    processes all 384 CLS tokens on its H-shard and emits a partial output
    summed on the host. Routing (gate logits/sigmoid/top-1 masks) is computed
    on the host (O(B*6*D), negligible) and shipped as {0,1}/weight masks
    folded into the device compute.

Key structure (v2):
  - CLS tokens are ordered slot-major (t = slot*64 + batch). Each slot has
    exactly TWO candidate atoms (left LEFT_KEYS//5, right RIGHT_KEYS//5), so
    the atom in/out GEMMs only compute 2 candidates per token (768 moving
    cols per d-tile instead of 1920): atoms 0-2 cover contiguous 128-col
    slot pairs, atoms 3/4 cover strided 3x64 slot sets.
  - PE order: patch chunk0 (piecewise w1 arrival) -> atom in-GEMM+select ->
    chunk1 -> chunk2 -> atom out-GEMM -> chunk3. Atom phases sit in the DMA
    shadow. Warm-up matmuls run during the DMA boot window so the PE p-state
    is ramped when real work arrives.
  - DMA: sync queue carries only the patch critical path (x0, w1 pieces, b1,
    x1); the gpsimd queue (cls/atom tensors, w2, x2/3) is held back behind a
    dummy Pool-engine dependency so it does not steal bandwidth from w1;
    vector queue carries all output stores.
"""

import numpy as np
import ml_dtypes

import concourse.bass as bass
import concourse.bacc as bacc
import concourse.mybir as mybir
from concourse import tile
from concourse.bass_utils import run_bass_kernel_spmd

NCORES = 8
B, NCLS, P, D, H = 64, 6, 196, 768, 3072
NA = 5
HSH = H // NCORES            # 384: per-core atom hidden shard
BPC = B // NCORES            # 8 batches per core
TPC = BPC * P                # 1568 patch tokens per core
NT = B * NCLS                # 384 cls tokens
DT = D // 128                # 6 d-tiles
HT = H // 128                # 24 h-tiles
HLT = NA * HSH // 128        # 15 atom h-shard tiles (a-major, 3 per atom)
KPA = HSH // 128             # 3 h-shard tiles per atom
CW = 392
NCH = 4

W1PIECES = [2, 2, 2, 3, 3, 3, 3, 3, 3]  # h-tiles per w1 load piece
W1QUEUE = [0, 1, 2, 0, 1, 2, 0, 1, 2]   # 0=sync, 1=scalar, 2=gpsimd
W1OFF = np.cumsum([0] + W1PIECES)
PIECE_OF_H = sum(([i] * n for i, n in enumerate(W1PIECES)), [])

LEFT_KEYS = np.array([3, 4, 8, 9, 13, 14], dtype=np.int64)
RIGHT_KEYS = np.array([15, 20, 16, 21, 17, 22], dtype=np.int64)

BF16 = mybir.dt.bfloat16
F32 = mybir.dt.float32
AF = mybir.ActivationFunctionType

_CACHE = {}
LAST_RESULTS = None  # BassKernelResults of the most recent run (for profiling)


def _build_program():
    nc = bacc.Bacc(None, target_bir_lowering=False, debug=False,
                   num_devices=NCORES)

    # partition-major packed inputs (see host layouts in kernel())
    xT_d = nc.dram_tensor("xT", [128, NCH * DT * CW], BF16,
                          kind="ExternalInput")
    w1T_d = nc.dram_tensor("w1T", [128, HT * DT * 128], BF16,
                           kind="ExternalInput")
    b1T_d = nc.dram_tensor("b1T", [128, HT], F32, kind="ExternalInput")
    w2T_d = nc.dram_tensor("w2T", [128, HT * D], BF16, kind="ExternalInput")
    clsT_d = nc.dram_tensor("clsT", [128, DT * NT], BF16,
                            kind="ExternalInput")
    ainT_d = nc.dram_tensor("ainT", [DT, 128, NA * HSH], BF16,
                            kind="ExternalInput")
    ainbT_d = nc.dram_tensor("ainbT", [128, HLT], F32, kind="ExternalInput")
    aoutT_d = nc.dram_tensor("aoutT", [NA, 128, KPA * D], BF16,
                             kind="ExternalInput")
    masks_d = nc.dram_tensor("masks", [128, 4 * NT], BF16,
                             kind="ExternalInput")
    poutT_d = nc.dram_tensor("poutT", [DT, 128, TPC], F32,
                             kind="ExternalOutput")
    cpartT_d = nc.dram_tensor("cpartT", [DT, 128, NT], F32,
                              kind="ExternalOutput")

    with tile.TileContext(nc) as tc:
        with (
            tc.tile_pool(name="w", bufs=1) as wp,
            tc.tile_pool(name="gat", bufs=1) as gp,
            tc.tile_pool(name="sel", bufs=1) as sp,
            tc.tile_pool(name="xin", bufs=4) as xp,
            tc.tile_pool(name="g1", bufs=24) as g1p,
            tc.tile_pool(name="ostg", bufs=3) as op,
            tc.tile_pool(name="ps", bufs=8, space="PSUM") as pp,
        ):
            # ---- patch critical path spread over all three DGE queues
            # (sync, scalar, gpsimd front; each sustains only ~200 B/ns),
            # pieces round-robin in consumption order ----
            # warm-up tile memset first so the PE can start ramping early
            wtile = wp.tile([128, 512], BF16, tag="warm", name="warm")
            nc.gpsimd.memset(wtile[:], 0)
            ENGS = [nc.sync, nc.scalar, nc.gpsimd]
            xs = [xp.tile([128, DT * CW], BF16, tag="x", name="x")]
            for q in range(3):
                ENGS[q].dma_start(xs[0][:, 2 * q * CW:2 * (q + 1) * CW],
                                  xT_d[:, 2 * q * CW:2 * (q + 1) * CW])
            b1T = wp.tile([128, HT], F32, tag="b1", name="b1")
            nc.scalar.dma_start(b1T[:], b1T_d[:])
            w1p = []
            for i, n in enumerate(W1PIECES):
                t = wp.tile([128, n * DT * 128], BF16, tag=f"w1p{i}",
                            name=f"w1p{i}")
                ENGS[W1QUEUE[i]].dma_start(
                    t[:], w1T_d[:, W1OFF[i] * DT * 128:W1OFF[i + 1] * DT * 128])
                w1p.append(t)
            xs.append(xp.tile([128, DT * CW], BF16, tag="x", name="x"))
            nc.sync.dma_start(xs[1][:], xT_d[:, DT * CW:2 * DT * CW])

            # ---- warm-up: ramp the PE p-state during the DMA boot window.
            # Independent psum tiles so the matmuls stream back-to-back. ----
            for i in range(10):
                wps = pp.tile([128, 512], F32, tag="ps", name=f"wps{i}")
                nc.tensor.matmul(wps[:, :512], wtile[:, :128], wtile[:, :512],
                                 start=True, stop=True)

            # ---- gpsimd queue: atom + later tensors, held behind chunk0's
            # first gelu so it does not compete with the w1 stream ----
            junk = wp.tile([128, CW], BF16, tag="junk", name="junk")
            gate_src = {}  # filled by patch_chunk(0): first g1 tile

            def gated_dma(t, src):
                # Pre-write the destination so the DMA carries a real data
                # dependency on the gate (the scheduler reorders engine
                # instructions by data deps, not program order).
                nc.gpsimd.tensor_copy(t[:, :64], junk[:, :64])
                nc.gpsimd.dma_start(t[:], src)

            def gpsimd_loads():
                nc.gpsimd.tensor_copy(junk[:], gate_src[0][:, :CW])
                w2T = [wp.tile([128, 12 * D], BF16, tag=f"w2{q}",
                               name=f"w2{q}") for q in range(2)]
                gated_dma(w2T[0], w2T_d[:, :12 * D])
                gated_dma(w2T[1], w2T_d[:, 12 * D:])
                clsT = wp.tile([128, DT * NT], BF16, tag="cls", name="cls")
                gated_dma(clsT, clsT_d[:])
                ainT = [wp.tile([128, NA * HSH], BF16, tag=f"ain{d}",
                                name=f"ain{d}") for d in range(DT)]
                for d in range(DT):
                    gated_dma(ainT[d], ainT_d[d])
                ainbT = wp.tile([128, HLT], F32, tag="ainb", name="ainb")
                nc.gpsimd.dma_start(ainbT[:], ainbT_d[:])
                masks = wp.tile([128, 4 * NT], BF16, tag="mask", name="mask")
                gated_dma(masks, masks_d[:])
                xs.append(load_x2(2))
                xs.append(load_x2(3))
                aoutT = [wp.tile([128, KPA * D], BF16, tag=f"ao{a}",
                                 name=f"ao{a}") for a in range(NA)]
                for a in range(NA):
                    gated_dma(aoutT[a], aoutT_d[a])
                return clsT, ainT, w2T, ainbT, masks, aoutT

            def load_x2(ci):
                xa = xp.tile([128, DT * CW], BF16, tag="x", name="x")
                nc.gpsimd.tensor_copy(xa[:, :64], junk[:, :64])
                nc.gpsimd.dma_start(
                    xa[:], xT_d[:, ci * DT * CW:(ci + 1) * DT * CW])
                return xa

            # ---- patch chunk: in-GEMM -> gelu -> out-GEMM -> store ----
            def patch_chunk(ci, xa, w2T, split_last=False):
                g1s = []
                for h in range(HT):
                    pi = PIECE_OF_H[h]
                    hh = h - W1OFF[pi]
                    ps = pp.tile([128, 512], F32, tag="ps", name="ps")
                    for d in range(DT):
                        c0 = (hh * DT + d) * 128
                        nc.tensor.matmul(ps[:, :CW], w1p[pi][:, c0:c0 + 128],
                                         xa[:, d * CW:(d + 1) * CW],
                                         start=(d == 0), stop=(d == DT - 1))
                    g1 = g1p.tile([128, CW], BF16, tag="g1", name="g1")
                    nc.scalar.activation(g1[:], ps[:, :CW], AF.Gelu,
                                         bias=b1T[:, h:h + 1])
                    g1s.append(g1)
                    if ci == 0 and h == 0:
                        gate_src[0] = g1
                        gate_src["atom"] = gpsimd_loads()
                if w2T is None:
                    w2T = gate_src["atom"][2]
                for dp in range(DT):
                    halves = [(0, CW)]
                    for (o, w) in halves:
                        ps = pp.tile([128, 512], F32, tag="ps", name="ps")
                        for h in range(HT):
                            c0 = (h % 12) * D + dp * 128
                            nc.tensor.matmul(ps[:, :w],
                                             w2T[h // 12][:, c0:c0 + 128],
                                             g1s[h][:, o:o + w],
                                             start=(h == 0), stop=(h == HT - 1))
                        stg = op.tile([128, CW], F32, tag="ostg", name="ostg")
                        nc.vector.tensor_copy(stg[:, :w], ps[:, :w])
                        nc.sync.dma_start(
                            poutT_d[dp][:, ci * CW + o:ci * CW + o + w],
                            stg[:, :w])

            patch_chunk(0, xs[0], None)
            clsT, ainT, w2T, ainbT, masks, aoutT = gate_src["atom"]

            # ---- atom in-GEMM + gelu (2 candidates per slot) ----
            # atoms 0-2: slot pair (2a, 2a+1) -> cols [a*128, (a+1)*128)
            # atoms 3/4: slots (0,2,4)/(1,3,5) -> strided 3x64 col blocks
            cls_r = clsT[:].rearrange("p (d s b) -> p d s b", d=DT, s=NCLS)
            gL = [gp.tile([128, NT], BF16, tag=f"gL{k}", name=f"gL{k}")
                  for k in range(KPA)]
            gR = [gp.tile([128, NT], BF16, tag=f"gR{k}", name=f"gR{k}")
                  for k in range(KPA)]
            for a in range(NA):
                for k in range(KPA):
                    ps = pp.tile([128, 512], F32, tag="ps", name="ps")
                    wcol = a * HSH + k * 128
                    for d in range(DT):
                        if a < 3:
                            mov = clsT[:, d * NT + a * 128:
                                       d * NT + (a + 1) * 128]
                            out = ps[:, :128]
                        else:
                            mov = cls_r[:, d, (a - 3):NCLS:2, :]
                            out = ps[:, :192].rearrange("p (c b) -> p c b",
                                                        c=3)
                        nc.tensor.matmul(out, ainT[d][:, wcol:wcol + 128],
                                         mov, start=(d == 0),
                                         stop=(d == DT - 1))
                    hl = a * KPA + k
                    if a < 3:
                        oap = gL[k][:, a * 128:(a + 1) * 128]
                        iap = ps[:, :128]
                    else:
                        oap = gR[k][:].rearrange(
                            "p (s b) -> p s b", s=NCLS)[:, (a - 3):NCLS:2, :]
                        iap = ps[:, :192].rearrange("p (c b) -> p c b", c=3)
                    nc.scalar.activation(oap, iap, AF.Gelu,
                                         bias=ainbT[:, hl:hl + 1])

            # ---- select + dst-weight masks (DVE, overlaps patch) ----
            mL, mR = masks[:, :NT], masks[:, NT:2 * NT]
            mwL, mwR = masks[:, 2 * NT:3 * NT], masks[:, 3 * NT:]
            hLs, hRs = [], []
            for k in range(KPA):
                t1 = sp.tile([128, NT], BF16, tag="t1", name="t1")
                t2 = sp.tile([128, NT], BF16, tag="t2", name="t2")
                sel = sp.tile([128, NT], BF16, tag=f"sel{k}", name=f"sel{k}")
                nc.vector.tensor_mul(t1[:], gL[k][:], mL)
                nc.vector.tensor_mul(t2[:], gR[k][:], mR)
                nc.vector.tensor_add(sel[:], t1[:], t2[:])
                hL = sp.tile([128, NT], BF16, tag=f"hL{k}", name=f"hL{k}")
                hR = sp.tile([128, NT], BF16, tag=f"hR{k}", name=f"hR{k}")
                nc.vector.tensor_mul(hL[:], sel[:], mwL)
                nc.vector.tensor_mul(hR[:], sel[:], mwR)
                hLs.append(hL)
                hRs.append(hR)

            patch_chunk(1, xs[1], w2T)
            patch_chunk(2, xs[2], w2T)

            # ---- atom out-GEMM: psL (atoms 0-2) + psR (atoms 3/4) ----
            for dp in range(DT):
                psL = pp.tile([128, 512], F32, tag="ps", name="ps")
                psR = pp.tile([128, 512], F32, tag="ps", name="ps")
                for a in range(3):
                    for k in range(KPA):
                        c0 = k * D + dp * 128
                        nc.tensor.matmul(psL[:, a * 128:(a + 1) * 128],
                                         aoutT[a][:, c0:c0 + 128],
                                         hLs[k][:, a * 128:(a + 1) * 128],
                                         start=(k == 0), stop=(k == KPA - 1))
                for a in (3, 4):
                    for k in range(KPA):
                        c0 = k * D + dp * 128
                        oap = psR[:, :NT].rearrange(
                            "p (s b) -> p s b", s=NCLS)[:, (a - 3):NCLS:2, :]
                        mov = hRs[k][:].rearrange(
                            "p (s b) -> p s b", s=NCLS)[:, (a - 3):NCLS:2, :]
                        nc.tensor.matmul(oap, aoutT[a][:, c0:c0 + 128], mov,
                                         start=(k == 0), stop=(k == KPA - 1))
                stg = op.tile([128, CW], F32, tag="ostg", name="ostg")
                nc.vector.tensor_copy(stg[:, :NT], psL[:, :NT])
                nc.vector.tensor_add(stg[:, :NT], stg[:, :NT], psR[:, :NT])
                nc.sync.dma_start(cpartT_d[dp], stg[:, :NT])

            patch_chunk(3, xs[3], w2T, split_last=True)

    nc.compile()
    return nc


def _sigmoid(x):
    out = np.empty_like(x)
    pos = x >= 0
    out[pos] = 1.0 / (1.0 + np.exp(-x[pos]))
    ex = np.exp(x[~pos])
    out[~pos] = ex / (1.0 + ex)
    return out


def kernel(x, patch_w1, patch_b1, patch_w2, patch_b2, gate_delta,
           atom_in_w, atom_in_b, atom_out_w, atom_out_b):
    x = np.asarray(x, dtype=np.float32)
    patch_w1 = np.asarray(patch_w1, dtype=np.float32)
    patch_b1 = np.asarray(patch_b1, dtype=np.float32)
    patch_w2 = np.asarray(patch_w2, dtype=np.float32)
    patch_b2 = np.asarray(patch_b2, dtype=np.float32)
    gate_delta = np.asarray(gate_delta, dtype=np.float32)
    atom_in_w = np.asarray(atom_in_w, dtype=np.float32)
    atom_in_b = np.asarray(atom_in_b, dtype=np.float32)
    atom_out_w = np.asarray(atom_out_w, dtype=np.float32)
    atom_out_b = np.asarray(atom_out_b, dtype=np.float32)

    bf = ml_dtypes.bfloat16

    # ---- host routing (tiny); slot-major token order t = n*64 + b ----
    cls3 = x[:, :NCLS, :]                                   # [B, 6, D]
    logits = np.einsum("bnd,nd->bn", cls3, gate_delta)      # [B, 6] f32
    choose_left = logits >= 0
    p_left = _sigmoid(logits)
    wgt = np.where(choose_left, p_left, 1.0 - p_left).astype(np.float32)
    keys = np.where(choose_left, LEFT_KEYS[None, :], RIGHT_KEYS[None, :])
    # slot-major flattening
    left_sm = choose_left.T.reshape(-1)                     # [384]
    w_sm = wgt.T.reshape(-1).astype(np.float32)             # [384]
    dst_sm = (keys % NA).T.reshape(-1)                      # [384]

    mL = left_sm.astype(np.float32)
    mR = 1.0 - mL
    mwL = mR * w_sm          # dst in {0,1,2} <=> right-chosen
    mwR = mL * w_sm          # dst in {3,4}   <=> left-chosen
    masks = np.ascontiguousarray(np.broadcast_to(
        np.concatenate([mL, mR, mwL, mwR]).reshape(1, 4 * NT),
        (128, 4 * NT))).astype(bf)

    # ---- replicated tensors (partition-major packed) ----
    # clsT[p, d*NT + n*64 + b] = cls3[b, n, d*128+p]
    clsT = np.ascontiguousarray(
        cls3.reshape(B, NCLS, DT, 128).transpose(3, 2, 1, 0)
    ).reshape(128, DT * NT).astype(bf)
    # w1T[p, (h*DT + d)*128 + m] = patch_w1[h*128+m, d*128+p]
    w1T = np.ascontiguousarray(
        patch_w1.reshape(HT, 128, DT, 128).transpose(3, 0, 2, 1)
    ).reshape(128, HT * DT * 128).astype(bf)
    b1T = np.ascontiguousarray(patch_b1.reshape(HT, 128).T)
    # w2T[p, h*D + dp*128 + m] = patch_w2[dp*128+m, h*128+p]
    w2T = np.ascontiguousarray(
        patch_w2.reshape(DT, 128, HT, 128).transpose(3, 2, 0, 1)
    ).reshape(128, HT * D).astype(bf)

    # ---- per-core tensors ----
    patch = x[:, NCLS:, :].reshape(NCORES, TPC, D)
    # xT[p, ci*DT*CW + d*CW + t] = patch[c][ci*CW+t, d*128+p]
    xT_all = np.ascontiguousarray(
        patch.reshape(NCORES, NCH, CW, DT, 128).transpose(0, 4, 1, 3, 2)
    ).reshape(NCORES, 128, NCH * DT * CW).astype(bf)

    ainT_all, ainbT_all, aoutT_all = [], [], []
    for c in range(NCORES):
        hsl = slice(HSH * c, HSH * (c + 1))
        # ainT[d, p, a*HSH + k*128 + m] = atom_in_w[a, hsl0 + k*128+m, d*128+p]
        ainT = np.ascontiguousarray(
            atom_in_w[:, hsl, :].reshape(NA, KPA, 128, DT, 128)
            .transpose(3, 4, 0, 1, 2)).reshape(DT, 128, NA * HSH).astype(bf)
        ainT_all.append(ainT)
        ainbT_all.append(np.ascontiguousarray(
            atom_in_b[:, hsl].reshape(HLT, 128).T))
        # aoutT[a, p, k*D + dp*128 + m] = atom_out_w[a, dp*128+m, hsl0+k*128+p]
        aoutT = np.ascontiguousarray(
            atom_out_w[:, :, hsl].reshape(NA, DT, 128, KPA, 128)
            .transpose(0, 4, 3, 1, 2)).reshape(NA, 128, KPA * D).astype(bf)
        aoutT_all.append(aoutT)

    in_maps = []
    for c in range(NCORES):
        in_maps.append({
            "xT": xT_all[c], "w1T": w1T, "b1T": b1T, "w2T": w2T,
            "clsT": clsT, "ainT": ainT_all[c], "ainbT": ainbT_all[c],
            "aoutT": aoutT_all[c], "masks": masks,
        })

    nc = _CACHE.get("nc")
    if nc is None:
        nc = _build_program()
        _CACHE["nc"] = nc

    res = run_bass_kernel_spmd(nc, in_maps, core_ids=list(range(NCORES)))
    global LAST_RESULTS
    LAST_RESULTS = res

    # ---- host gather ----
    patch_out = np.empty((B, P, D), dtype=np.float32)
    for c in range(NCORES):
        poutT = res.results[c]["poutT"].reshape(D, TPC)
        patch_out[BPC * c:BPC * (c + 1)] = (
            poutT.T + patch_b2[None, :]).reshape(BPC, P, D)

    cpart = np.zeros((D, NT), dtype=np.float32)
    for c in range(NCORES):
        cpart += res.results[c]["cpartT"].reshape(D, NT)
    cls_sm = cpart.T + w_sm[:, None] * atom_out_b[dst_sm, :]   # [384, D]
    cls_out = cls_sm.reshape(NCLS, B, D).transpose(1, 0, 2)

    return np.concatenate([cls_out, patch_out], axis=1)
